# revision 9
# baseline (speedup 1.0000x reference)
"""MFE loss kernel for Trainium2 (8 NeuronCores).

Math (per sample i, with o = others_idx):
    p = softmax(preds[i]);  w = p[o]
    fne_i ~= (1 - w)^2   (for samples with target == o)
    fpe_i = w^2          (for the rest)
    out = mean(fne_i | g) + mean(fpe_i | ~g)

Sharding strategy ("partition by class" instead of mask-on-device):
the host groups samples by g = (target == o): all g=1 rows first, then
g=0 rows, and converts preds to bf16 (RNE).  The device then needs NO
target tensor and NO mask ops - per sample it computes only
    E = exp(x)  (bf16)
    S = e0+e1+e2+e3 ; R ~= 1/S ; w = e_o * R ; q = w^2
and accumulates Sum(w), Sum(q) per (partition, tile) cell.  Each cell
covers a known contiguous range of sorted rows, so the host classifies
cells into the g=1 / g=0 region; the single cell straddling the region
boundary is recomputed on the host in f64 (<= 1024 of 8.4M samples).
Final: fne = (n1 - 2*S1w + S1q)/n1, fpe = S0q/(B-n1), out = fne+fpe.

Engine budget per core (1.05M samples, per-sample elems):
    ACT   exp x4 (+ Square on small tiles)       ~28 us
    DVE   pairadd(bf16 2x) + recip + w-mult + w^2 ~26-30 us
    GPSIMD the strided S-add (Add eff 0.42)       ~16 us
    DMA   8 B/sample bf16                         ~25 us
vs the f32+mask baseline which measured 100.5 us (DMA 76, DVE 68, ACT 68).
"""

import os
import sys

import numpy as np
import ml_dtypes

for _p in ("/opt/trn_rl_repo", "/root/.axon_site/_ro/trn_rl_repo"):
    if _p not in sys.path and os.path.isdir(_p):
        sys.path.append(_p)

B = 8388608
C = 4
N_CORES = 8
BC = B // N_CORES          # 1048576 samples per core
P = 128                    # SBUF partitions
FD = BC // P               # 8192 samples per partition per core
F_SCHED = [512] + [1024] * 7 + [256, 256]
assert sum(F_SCHED) == FD
N_TILES = len(F_SCHED)
F_OFF = [sum(F_SCHED[:i]) for i in range(N_TILES)]
# Per-tile engine for the w^2+accum op, to balance ACT/DVE load
# (GPSIMD rejects scalar_tensor_tensor at codegen):
# 'd' = DVE stt, 'a' = ACT Square.
SQ_ENGINE = ["d", "d", "a", "d", "a", "d", "a", "d", "a", "a"]

BF16 = ml_dtypes.bfloat16

_BUILD_CACHE = {}


def _build(others_idx: int):
    """Build + compile the Bass program (shared by all 8 cores)."""
    from contextlib import ExitStack

    import concourse.bass as bass  # noqa: F401
    import concourse.tile as tile
    from concourse import bacc, mybir

    f32 = mybir.dt.float32
    bf16 = mybir.dt.bfloat16
    Alu = mybir.AluOpType
    Act = mybir.ActivationFunctionType

    nc = bacc.Bacc(
        "TRN2", target_bir_lowering=False, debug=False, num_devices=N_CORES
    )

    preds = nc.dram_tensor("preds", (BC, C), bf16, kind="ExternalInput").ap()
    acc_w = nc.dram_tensor("accw", (P, N_TILES), f32, kind="ExternalOutput").ap()
    acc_q = nc.dram_tensor("accq", (P, N_TILES), f32, kind="ExternalOutput").ap()

    # Partition-major layout: sample = p*FD + n, so each partition's
    # slice of a tile is one contiguous run in HBM.
    xv = preds.rearrange("(p n) c -> p n c", p=P)     # [128, 8192, 4]

    oi = int(others_idx)

    with ExitStack() as ctx:
        tc = ctx.enter_context(tile.TileContext(nc))
        xp = ctx.enter_context(tc.tile_pool(name="x", bufs=6))
        ep = ctx.enter_context(tc.tile_pool(name="e", bufs=6))
        up = ctx.enter_context(tc.tile_pool(name="u", bufs=5))
        sp = ctx.enter_context(tc.tile_pool(name="s", bufs=5))
        rp = ctx.enter_context(tc.tile_pool(name="r", bufs=5))
        wp = ctx.enter_context(tc.tile_pool(name="w", bufs=4))
        qp = ctx.enter_context(tc.tile_pool(name="q", bufs=4))
        accp = ctx.enter_context(tc.tile_pool(name="acc", bufs=1))

        a_w = accp.tile([P, N_TILES], f32)
        a_q = accp.tile([P, N_TILES], f32)

        for i, fi in enumerate(F_SCHED):
            off = F_OFF[i]
            xt = xp.tile([P, fi * C], bf16, tag="x")
            nc.sync.dma_start(xt[:], xv[:, off : off + fi, :])

            et = ep.tile([P, fi * C], bf16, tag="e")
            nc.scalar.activation(et[:], xt[:], Act.Exp)
            ev = et[:].rearrange("p (n c) -> p n c", c=C)

            # Pairwise class sum: bf16 packed pairs -> DVE 2x mode.
            u = up.tile([P, 2 * fi], bf16, tag="u")
            uv = u[:].rearrange("p (n c) -> p n c", c=2)
            nc.vector.tensor_add(uv, ev[:, :, 0:2], ev[:, :, 2:4])

            # Second (strided) add on the otherwise-idle GPSIMD engine.
            s = sp.tile([P, fi], f32, tag="s")
            nc.gpsimd.tensor_add(s[:], uv[:, :, 0], uv[:, :, 1])

            r = rp.tile([P, fi], f32, tag="r")
            nc.vector.reciprocal_approx_fast(r[:], s[:])

            w = wp.tile([P, fi], bf16, tag="w")
            nc.vector.scalar_tensor_tensor(
                w[:], ev[:, :, oi], 1.0, r[:],
                op0=Alu.mult, op1=Alu.mult,
                accum_out=a_w[:, i : i + 1],
            )

            q = qp.tile([P, fi], bf16, tag="q")
            eng = SQ_ENGINE[i]
            if eng == "a":
                nc.scalar.activation(
                    q[:], w[:], Act.Square, accum_out=a_q[:, i : i + 1]
                )
            else:
                nc.vector.scalar_tensor_tensor(
                    q[:], w[:], 1.0, w[:],
                    op0=Alu.mult, op1=Alu.mult,
                    accum_out=a_q[:, i : i + 1],
                )

        nc.sync.dma_start(acc_w, a_w[:])
        nc.sync.dma_start(acc_q, a_q[:])

    nc.compile()
    return nc


def _get_nc(others_idx: int):
    key = int(others_idx)
    if key not in _BUILD_CACHE:
        _BUILD_CACHE[key] = _build(key)
    return _BUILD_CACHE[key]


def _to_bf16_rne(x: np.ndarray) -> np.ndarray:
    """f32 -> bf16 with round-to-nearest-even, via integer bit math."""
    v = x.view(np.uint32)
    r = ((v + np.uint32(0x7FFF) + ((v >> np.uint32(16)) & np.uint32(1)))
         >> np.uint32(16)).astype(np.uint16)
    return r.view(BF16)


def _prepare(preds: np.ndarray, target: np.ndarray, oi: int):
    """Host prep: bf16-cast preds, group rows by g=(target==oi) (g=1 rows
    first), return (per-core input maps, device-order bf16 array, n1)."""
    preds = np.asarray(preds)
    if preds.dtype != np.float32:
        preds = preds.astype(np.float32)
    target = np.asarray(target)
    g = target == oi
    n1 = int(np.count_nonzero(g))

    xb = _to_bf16_rne(preds)
    dev = np.empty((B, C), dtype=BF16)
    dev[:n1] = xb[g]
    dev[n1:] = xb[~g]

    in_maps = [
        {"preds": dev[c * BC : (c + 1) * BC]} for c in range(N_CORES)
    ]
    return in_maps, dev, n1


def _combine(results, dev: np.ndarray, n1: int, oi: int):
    """Classify (core, partition, tile) cells against the region boundary
    n1, sum the device accumulators per region, fix up the one straddling
    cell on the host, and finish the loss."""
    # Global start row of every cell.
    offs = np.asarray(F_OFF)                                   # [NT]
    starts = (
        np.arange(N_CORES)[:, None, None] * BC
        + np.arange(P)[None, :, None] * FD
        + offs[None, None, :]
    )                                                          # [8,128,NT]
    lens = np.asarray(F_SCHED)[None, None, :]
    ends = starts + lens

    accw = np.stack([np.asarray(r["accw"], dtype=np.float64) for r in results])
    accq = np.stack([np.asarray(r["accq"], dtype=np.float64) for r in results])

    in1 = ends <= n1                    # fully in region 1 (target == oi)
    in0 = starts >= n1                  # fully in region 0
    s1w = float(accw[in1].sum())
    s1q = float(accq[in1].sum())
    s0q = float(accq[in0].sum())

    straddle = ~(in1 | in0)
    if straddle.any():
        ci, pi, ti = (int(v) for v in np.argwhere(straddle)[0])
        g0 = int(starts[ci, pi, ti])
        fi = int(F_SCHED[int(ti)])
        rows = dev[g0 : g0 + fi].astype(np.float64)
        e = np.exp(rows)
        w = e[:, oi] / e.sum(axis=1)
        k = n1 - g0                     # first k rows of the cell are region 1
        s1w += float(w[:k].sum())
        s1q += float((w[:k] ** 2).sum())
        s0q += float((w[k:] ** 2).sum())

    fne = (n1 - 2.0 * s1w + s1q) / n1
    fpe = s0q / (B - n1)
    return np.asarray(np.float32(fne + fpe))


def kernel(preds, target, others_idx):
    from concourse import bass_utils

    oi = int(np.asarray(others_idx))
    nc = _get_nc(oi)
    in_maps, dev, n1 = _prepare(preds, target, oi)
    res = bass_utils.run_bass_kernel_spmd(
        nc, in_maps, core_ids=list(range(N_CORES))
    )
    return _combine(res.results, dev, n1, oi)


if __name__ == "__main__":
    rng = np.random.default_rng(0)
    preds = rng.standard_normal((B, C), dtype=np.float32)
    target = rng.integers(0, C, size=(B,), dtype=np.int64)
    out = kernel(preds, target, 3)
    print("kernel out:", out, out.dtype, out.shape)
